# revision 6
# baseline (speedup 1.0000x reference)
import numpy as np

# nn_GraphTransformerDemon: B=4, S=384, IN=32, H=64, NH=4
# Sharding: 8 cores = (batch b, i-half) pairs; each core handles 192 i-rows x 384 j
# of the pair grid for its batch. Only pooled sums are needed:
#   SA[h]  = sum_ij relu(L_i + R_j)[h]
#   SAK[h] = sum_ij keep_ij * relu(L_i + R_j)[h]
#   SK     = sum_ij keep_ij
# with keep_ij = sigmoid(sum_h' Wd2[h'] * relu(dL_i + dR_j)[h'] + bd2).
# msgs@We2 is folded algebraically on the host afterwards.

B, S, IN, H, NH, DH, NC_ = 4, 384, 32, 64, 4, 16, 3
ISH = S // 2            # 192 i-rows per core
NJT = S // 128          # 3 j-tiles
NCE = ISH // 8          # 24 eh chunks (8 i x 64 h = 512)
NCD = ISH // 16         # 12 dh chunks (16 i x 32 h = 512)

_BUILT = {}


def _build(bd2f):
    import concourse.bass as bass
    import concourse.bacc as bacc
    import concourse.mybir as mybir
    from concourse import tile

    f32 = mybir.dt.float32
    AF = mybir.ActivationFunctionType
    AL = mybir.AluOpType
    AX = mybir.AxisListType

    nc = bacc.Bacc("TRN2", target_bir_lowering=False, debug=False, num_devices=8)
    rt_d = nc.dram_tensor("rt", [H + 1, S], f32, kind="ExternalInput")
    drt_d = nc.dram_tensor("drt", [33, S], f32, kind="ExternalInput")
    lf_d = nc.dram_tensor("lf", [1, ISH * H], f32, kind="ExternalInput")
    dlf_d = nc.dram_tensor("dlf", [1, ISH * 32], f32, kind="ExternalInput")
    ip64_d = nc.dram_tensor("ip64", [64, 512], f32, kind="ExternalInput")
    ip32_d = nc.dram_tensor("ip32", [32, 512], f32, kind="ExternalInput")
    wd2_d = nc.dram_tensor("wd2rep", [128, 512], f32, kind="ExternalInput")
    red_d = nc.dram_tensor("red", [8, 256], f32, kind="ExternalOutput")
    ksum_d = nc.dram_tensor("ksum", [128, 4], f32, kind="ExternalOutput")

    with tile.TileContext(nc) as tc:
        with (
            tc.tile_pool(name="const", bufs=1) as cp,
            tc.tile_pool(name="work", bufs=4) as wp,
            tc.tile_pool(name="psg", bufs=2, space="PSUM") as psg,
            tc.tile_pool(name="pse", bufs=3, space="PSUM") as pse,
            tc.tile_pool(name="psr", bufs=1, space="PSUM") as psr,
        ):
            rt = cp.tile([H + 1, S], f32)
            nc.sync.dma_start(rt[:], rt_d[:])
            drt = cp.tile([33, S], f32)
            nc.sync.dma_start(drt[:], drt_d[:])
            wd2 = cp.tile([128, 512], f32)
            nc.sync.dma_start(wd2[:], wd2_d[:])
            ip64 = cp.tile([64, 512], f32)
            nc.sync.dma_start(ip64[:], ip64_d[:])
            ip32 = cp.tile([32, 512], f32)
            nc.sync.dma_start(ip32[:], ip32_d[:])

            # fill rows [0:32) / [0:64) with the identity pattern replicated
            # NCD / NCE times, via chained doubling DMAs (few sync waits).
            rhs_dh = cp.tile([33, NCD * 512], f32)
            nc.sync.dma_start(rhs_dh[0:32, 0:512], ip32[:])
            w = 512
            while w < NCD * 512:
                n = min(w, NCD * 512 - w)
                nc.sync.dma_start(rhs_dh[0:32, w:w + n], rhs_dh[0:32, 0:n])
                w += n
            nc.sync.dma_start(rhs_dh[32:33, :], dlf_d[:])
            rhs_eh = cp.tile([H + 1, NCE * 512], f32)
            nc.sync.dma_start(rhs_eh[0:64, 0:512], ip64[:])
            w = 512
            while w < NCE * 512:
                n = min(w, NCE * 512 - w)
                nc.sync.dma_start(rhs_eh[0:64, w:w + n], rhs_eh[0:64, 0:n])
                w += n
            nc.sync.dma_start(rhs_eh[64:65, :], lf_d[:])

            keep = [cp.tile([128, 2 * ISH], f32, tag=f"keep{j}", name=f"keep{j}") for j in range(NJT)]
            klog = [cp.tile([128, ISH], f32, tag=f"klog{j}", name=f"klog{j}") for j in range(NJT)]
            ksum = cp.tile([128, 4], f32)
            nc.gpsimd.memset(ksum[:], 0.0)
            red_ps = psr.tile([8, 256], f32)

            nmm = NJT * NCE * 2
            mm = 0
            for jt in range(NJT):
                nc.gpsimd.memset(keep[jt][:], 1.0)
                for c in range(NCD):
                    pd = psg.tile([128, 512], f32, tag="pd")
                    nc.tensor.matmul(
                        pd[:], drt[:, jt * 128:(jt + 1) * 128],
                        rhs_dh[:, c * 512:(c + 1) * 512], start=True, stop=True)
                    dhw = wp.tile([128, 512], f32, tag="dhw")
                    nc.vector.scalar_tensor_tensor(
                        out=dhw[:], in0=pd[:], scalar=0.0, in1=wd2[:],
                        op0=AL.max, op1=AL.mult)
                    nc.vector.tensor_reduce(
                        out=klog[jt][:, c * 16:(c + 1) * 16],
                        in_=dhw[:].rearrange("p (i h) -> p i h", h=32),
                        axis=AX.X, op=AL.add)
                # sigmoid into even columns of keep pairs: (k_i, 1) interleaved
                kview = keep[jt][:].rearrange("p (i two) -> p two i", two=2)
                nc.scalar.activation(
                    kview[:, 0, :], klog[jt][:], AF.Sigmoid, bias=float(bd2f))
                nc.vector.tensor_reduce(
                    out=ksum[:, jt:jt + 1], in_=kview[:, 0, :],
                    axis=AX.X, op=AL.add)
                for c in range(NCE):
                    pe_ = pse.tile([128, 512], f32, tag="pe")
                    nc.tensor.matmul(
                        pe_[:], rt[:, jt * 128:(jt + 1) * 128],
                        rhs_eh[:, c * 512:(c + 1) * 512], start=True, stop=True)
                    eh = wp.tile([128, 512], f32, tag="eh")
                    nc.scalar.activation(eh[:], pe_[:], AF.Relu)
                    for hf in range(2):
                        nc.tensor.matmul(
                            red_ps[:],
                            keep[jt][:, c * 16 + hf * 8: c * 16 + hf * 8 + 8],
                            eh[:, hf * 256:(hf + 1) * 256],
                            start=(mm == 0), stop=(mm == nmm - 1))
                        mm += 1

            red_sb = cp.tile([8, 256], f32)
            nc.vector.tensor_copy(red_sb[:], red_ps[:])
            nc.sync.dma_start(red_d[:], red_sb[:])
            nc.sync.dma_start(ksum_d[:], ksum[:])
    nc.compile()
    return nc


def _host_nodes(d):
    x = d["x"]
    n = np.maximum(x @ d["Wp"] + d["bp"], 0.0)
    q = (n @ d["Wq"] + d["bq"]).reshape(B, S, NH, DH)
    k = (n @ d["Wk"] + d["bk"]).reshape(B, S, NH, DH)
    v = (n @ d["Wv"] + d["bv"]).reshape(B, S, NH, DH)
    sc = np.einsum("bqhd,bkhd->bhqk", q, k) / np.float32(np.sqrt(DH))
    sc = sc - sc.max(-1, keepdims=True)
    e = np.exp(sc)
    a = e / e.sum(-1, keepdims=True)
    att = np.einsum("bhqk,bkhd->bqhd", a, v).reshape(B, S, H) @ d["Wo"] + d["bo"]

    def ln(t, g, b):
        m = t.mean(-1, keepdims=True)
        vv = ((t - m) ** 2).mean(-1, keepdims=True)
        return (t - m) / np.sqrt(vv + np.float32(1e-5)) * g + b

    n = ln(n + att, d["g1"], d["b1"])
    ff = np.maximum(n @ d["Wf1"] + d["bf1"], 0.0) @ d["Wf2"] + d["bf2"]
    return ln(n + ff, d["g2"], d["b2"]).astype(np.float32)


def kernel(**inputs):
    d = {k: np.asarray(v, dtype=np.float32) for k, v in inputs.items()}
    nodes = _host_nodes(d)
    We1, be1, We2, be2 = d["We1"], d["be1"], d["We2"], d["be2"]
    Wd1, bd1, Wd2, bd2 = d["Wd1"], d["bd1"], d["Wd2"], d["bd2"]

    if "nc" not in _BUILT:
        _BUILT["nc"] = _build(float(bd2[0]))
    nc = _BUILT["nc"]

    ip64 = np.tile(np.eye(64, dtype=np.float32), (1, 8)).copy()
    ip32 = np.tile(np.eye(32, dtype=np.float32), (1, 16)).copy()
    wd2rep = np.tile(Wd2[:, 0], (128, 16)).astype(np.float32)

    in_maps = []
    for core in range(8):
        b, ih = core // 2, core % 2
        L = nodes[b] @ We1[:H] + be1
        R = nodes[b] @ We1[H:]
        dL = nodes[b] @ Wd1[:H] + bd1
        dR = nodes[b] @ Wd1[H:]
        one = np.ones((1, S), np.float32)
        in_maps.append({
            "rt": np.concatenate([R.T, one], 0).astype(np.float32).copy(),
            "drt": np.concatenate([dR.T, one], 0).astype(np.float32).copy(),
            "lf": L[ih * ISH:(ih + 1) * ISH].reshape(1, -1).astype(np.float32).copy(),
            "dlf": dL[ih * ISH:(ih + 1) * ISH].reshape(1, -1).astype(np.float32).copy(),
            "ip64": ip64, "ip32": ip32, "wd2rep": wd2rep,
        })

    from concourse.bass_utils import run_bass_kernel_spmd
    import time as _time
    t0 = _time.perf_counter()
    r = run_bass_kernel_spmd(nc, in_maps, list(range(8)))
    _BUILT["dev_ns"] = (_time.perf_counter() - t0) * 1e9
    _BUILT["last"] = r
    res = r.results

    out = np.zeros((B, NC_), np.float32)
    g = np.arange(4)
    for b in range(B):
        SA = np.zeros(H, np.float32)
        SAK = np.zeros(H, np.float32)
        SK = np.float32(0.0)
        for ih in range(2):
            r = res[2 * b + ih]["red"]
            SAK += sum(r[2 * gg, 64 * gg:64 * (gg + 1)] for gg in g)
            SA += r[1].reshape(4, 64).sum(0)
            SK += r.dtype.type(0) + res[2 * b + ih]["ksum"][:, 0:3].sum()
        pa = nodes[b].mean(0) + (SAK @ We2 + SK * be2) / np.float32(S)
        pt = ((SA - SAK) @ We2 + (np.float32(S * S) - SK) * be2) / np.float32(S)
        h = np.maximum(np.concatenate([pa, pt]) @ d["Wc1"] + d["bc1"], 0.0)
        out[b] = h @ d["Wc2"] + d["bc2"]
    return out.astype(np.float32)
